# revision 2
# baseline (speedup 1.0000x reference)
"""Trainium2 Bass kernel for nn_HEMoETorch_43722767073393 (moe_routing) — v3 fp8.

Reference computation:
    h        = embed[x]                                  (N=4096, D=1024)
    h_fast   = relu(h @ fast_w1.T)
    scores   = exp(-max(||h-mu||^2, 0) / (2*sigma^2)) * charge     (N, 64)
    top_idx  = top_k(scores.mean(0), 8); top_w = scores[:, top_idx]
    slow_out = sum_k top_w[:,k] * (h @ expert_w[top_idx[k]].T)
    out      = (h_fast + 0.3 * slow_out) @ fast_w2.T     (N, 50257)

Numerical structure exploited: with D=1024, ||h - mu||^2 is ~1280 +- 60 for
every (token, expert) pair, so exp(-sq/8) < 1e-55 underflows to exactly 0.0
in fp32 for ALL pairs.  Hence top_w == 0 and slow_out == 0 *exactly* in the
fp32 reference, and the output is exactly relu(embed[x] @ W1^T) @ W2^T.
The host computes hm = relu(h @ W1^T) in fp32 (exact); the device runs the
single large vocab matmul  out^T[vocab_shard, tokens] = w2_shard @ hm^T.

v3: fp8 (float8e4 = e4m3) matmuls in MatmulPerfMode.DoubleRow, which
contracts K=256 per pass at 0.5 cycles per output column — 4x the bf16 MAC
rate.  e4m3 alone is too noisy (max-rel-err 3.8e-2 vs the 2e-2 gate), so
both operands are split hi+lo with error feedback and three DoubleRow
term-passes are accumulated in fp32 PSUM:

    out = hm_hi@w2_hi + hm_lo@w2_hi + hm_hi@w2_lo        (drop lo*lo)

at 0.75x the bf16 cycle count (measured rel err 3.2e-3, better than the
bf16 kernel's 2.6e-3+bf16 noise).  GH_KT/GW_KT knobs trim the correction
terms' k-coverage to trade accuracy for speed.  Output is drained to SBUF
as bf16 (halves output DMA) on alternating Vector/Scalar engines and
rescaled by 2^-12 on the host (power-of-two scales SH=16, SW=256 keep all
rounding exact).

Device strategy (8 NeuronCores, no collectives):
  - vocab-sharded: each core computes logits[:, shard] for 6283 vocab rows
    (padded to 6400 = 50 x 128).
  - token dedup: only unique token ids are computed (4096 -> ~3925, padded
    to 16*CW columns); duplicate rows are expanded on the host.
  - DoubleRow moving free dim is 2*CW <= 512, so tokens go in 16 chunks of
    CW=248; PSUM zero-regions are 2KB (one bank) so the 16 chunks run as
    two half-passes of 8 banks.
  - stationary-major streaming: each (w-version, ktile) 128x2x128 block
    streams all 8 chunks before switching; post-compile dedup_ldweights
    removes redundant InstLdweights.
  - post-compile strip_midchain_incs removes per-matmul semaphore
    increments from mid-chain matmuls and remaps wait thresholds.
  - reps>1 (timing) unrolls the body x2 with two h^T buffer sets so the
    next iteration's h^T DMA overlaps compute.
"""

import numpy as np
import ml_dtypes

import concourse.bass as bass  # noqa: F401  (bass must import before bacc)
import concourse.mybir as mybir
import concourse.tile as tile
import concourse.tile_rust as tile_rust
from concourse import bacc
from concourse.bass_utils import run_bass_kernel_spmd

BF16 = ml_dtypes.bfloat16
E4M3 = ml_dtypes.float8_e4m3   # mybir.dt.np(float8e4)

N_CORES = 8
B, S = 4, 1024
N = B * S            # 4096 tokens
D = 1024
V = 50257
VS = 6283            # ceil(V / 8); padded total = 50264
KT = D // 256        # 4 DoubleRow contraction tiles (256 logical d each)
VB = 50              # ceil(VS/128): 128-wide vocab blocks (padded to 6400)
NCHUNK = 16          # token chunks (2 half-passes x 8 PSUM banks)
SIGMA = 2.0
FAST_RATIO = 0.7
TOP_K = 8

SH = np.float32(16.0)     # hm quantization scale (power of two)
SW = np.float32(256.0)    # w2 quantization scale (power of two)
INV_S = np.float32(1.0 / 4096.0)

# correction-term k-coverage (in DoubleRow ktiles of 256, out of KT=4):
# GH_KT: hm_lo @ w2_hi term;  GW_KT: hm_hi @ w2_lo term.
GH_KT = 4
GW_KT = 4

_prog_cache: dict = {}
_N_ACTIVE = N          # padded token-column count; set by prepare_inputs
_CW = 256              # token chunk width = _N_ACTIVE / 16


def dedup_ldweights(nc):
    """Remove InstLdweights whose stationary AP is identical to the previous
    ldweights in the same basic block (the PE weight registers still hold the
    same values, so the reload is redundant).  Only sync-free ldweights are
    removed; ones carrying semaphore waits/updates are kept (and reset the
    tracked key so pairing stays conservative)."""
    removed = 0
    moved = 0
    kept = 0
    for bb in nc.m.functions[0].blocks:
        insts = list(bb.instructions)
        newlist = []
        last_key = None
        for idx, ins in enumerate(insts):
            if isinstance(ins, mybir.InstLdweights):
                key = (
                    str(ins.ins[0]),
                    str(ins.is_transpose),
                    str(ins.perf_mode),
                    str(ins.tile_position),
                )
                si = ins.sync_info
                if key == last_key and si is None:
                    removed += 1
                    continue
                if key == last_key and si is not None:
                    # redundant load kept only for its sync: transfer a
                    # single wait to the following matmul (which may hold
                    # at most one wait) and drop the load
                    nxt = insts[idx + 1] if idx + 1 < len(insts) else None
                    if (len(si.on_update) == 0 and len(si.on_wait) == 1
                            and isinstance(nxt, mybir.InstMatmult)
                            and not nxt.has_wait()):
                        nsi = nxt.sync_info
                        if nsi is None:
                            nxt.sync_info = mybir.SyncInfo(
                                on_wait=list(si.on_wait), on_update=[])
                        else:
                            nsi.on_wait = list(si.on_wait)
                        moved += 1
                        continue
                    kept += 1
                    newlist.append(ins)
                    continue
                last_key = key
                kept += 1
                newlist.append(ins)
            else:
                newlist.append(ins)
        if removed or moved:
            try:
                bb.instructions = newlist
            except Exception:
                live = bb.instructions
                del live[:]
                for i2 in newlist:
                    live.append(i2)
    print(f"dedup_ldweights: removed {removed}, moved {moved}, kept {kept}")
    return nc


def _resets_sem(ins, sid):
    """True if `ins` clears/resets semaphore id `sid`."""
    for attr_lo, attr_hi in (("reset_range_start", "reset_range_stop"),
                             ("range_first", "range_last")):
        lo = getattr(ins, attr_lo, None)
        hi = getattr(ins, attr_hi, None)
        if lo is not None and hi is not None and lo <= sid <= hi + 1:
            if getattr(ins, "is_reset_sema", False) or attr_lo == "range_first":
                return True
    return False


def strip_midchain_incs(nc, thin_stops: bool = False):
    """Remove semaphore increments from mid-chain matmuls (start/stop=False)
    and remap every wait threshold on the affected semaphore to the next
    surviving increment (a later completion -- conservative).

    Function-global, epoch-aware: sem-clear instructions split the stream
    into epochs; each wait is remapped against the increments of its own
    epoch.  Only semaphores whose increments all come from one engine are
    touched (in-order queue: count order == program order)."""
    stripped = 0
    fn = nc.m.functions[0]
    insts = [ins for bb in fn.blocks for ins in bb.instructions]

    # candidate sems: inc'd (value 1) by mid-chain matmuls anywhere
    has_mm_inc: set = set()
    for ins in insts:
        si = ins.sync_info
        if si is None:
            continue
        for u in si.on_update:
            if (u.sync_type == "semaphore" and u.update_mode == "sem-inc"
                    and isinstance(ins, mybir.InstMatmult)
                    and not ins.stop_tensor_calc and u.update_value == 1):
                has_mm_inc.add(u.id)
    cands = sorted(has_mm_inc)

    for sid in cands:
        # epoch-split pass; waits whose threshold exceeds the current epoch's
        # inc count so far are loop-wraparound waits (they reference the
        # previous iteration's body = the largest epoch) -> deferred
        epochs = []
        cur = {"incs": [], "waits": []}
        deferred = []          # (ins, widx, v)
        rebases = []           # (ins, uidx, value)  sem-add/sub loop rebases
        ok = True
        for ins in insts:
            if _resets_sem(ins, sid):
                epochs.append(cur)
                cur = {"incs": [], "waits": []}
                continue
            si = ins.sync_info
            if si is None:
                continue
            for widx, w in enumerate(si.on_wait):
                if w.sync_type == "semaphore" and w.id == sid:
                    if (w.wait_mode != "sem-ge-imm" or w.wait_value is None
                            or w.wait_value < 0):
                        ok = False
                        break
                    if w.wait_value == 0:
                        continue          # trivially satisfied, keep as-is
                    if w.wait_value > len(cur["incs"]):
                        deferred.append((ins, widx, w.wait_value))
                    else:
                        cur["waits"].append((ins, widx, w.wait_value))
            if not ok:
                break
            for uidx, u in enumerate(si.on_update):
                if u.sync_type == "semaphore" and u.id == sid:
                    if (u.update_mode in ("sem-add-imm", "sem-sub-imm")
                            and u.update_value > 1):
                        # loop rebase by the body's inc total: epoch boundary
                        rebases.append((ins, uidx, u.update_value))
                        epochs.append(cur)
                        cur = {"incs": [], "waits": []}
                        continue
                    if u.update_mode != "sem-inc" or u.update_value != 1:
                        ok = False
                        break
                    is_mm = isinstance(ins, mybir.InstMatmult)
                    strippable = is_mm and not ins.stop_tensor_calc
                    cur["incs"].append((ins, strippable,
                                        is_mm and ins.stop_tensor_calc))
                    cur.setdefault("engines", set()).add(str(ins.engine))
            if not ok:
                break
        if not ok:
            continue
        epochs.append(cur)
        # an epoch is strippable only if all its incs come from one engine
        # (in-order queue => count order == program order); otherwise keep
        # every inc in that epoch
        for ep in epochs:
            if len(ep.get("engines", set())) > 1:
                ep["incs"] = [(i2, False, st) for (i2, _, st) in ep["incs"]]
        big = max(epochs, key=lambda ep: len(ep["incs"]))
        if any(v > len(big["incs"]) for (_, _, v) in deferred):
            continue
        # every rebase amount must equal the body epoch's inc total
        if any(v != len(big["incs"]) for (_, _, v) in rebases):
            continue
        big["waits"] = big["waits"] + deferred

        new_wait_val: dict = {}    # (id(ins), widx) -> new value
        new_upd_val: dict = {}     # (id(ins), uidx) -> new value (rebases)
        strip_insts: set = set()   # id(ins) whose inc on sid is dropped
        for ep in epochs:
            incs, wlist = ep["incs"], ep["waits"]
            total = len(incs)
            if total == 0:
                continue
            kept = [not s for _, s, _ in incs]
            if thin_stops:
                # strip every other stop-matmul increment within each run of
                # consecutive kept stop incs (waiters round up to the next
                # kept one, at most one matmul later); always keep the last
                # of a run (DMA waits target it)
                i2 = 0
                while i2 < total:
                    if kept[i2] and incs[i2][2]:
                        run_start = i2
                        while i2 < total and kept[i2] and incs[i2][2]:
                            i2 += 1
                        for k3 in range(run_start, i2 - 1):
                            if (k3 - run_start) % 2 == 0:
                                kept[k3] = False
                    else:
                        i2 += 1
            for (_, _, v) in wlist:
                if not any(kept[v - 1:]):
                    kept[v - 1] = True
            pref = [0]
            for k in kept:
                pref.append(pref[-1] + (1 if k else 0))
            for (wins, widx, v) in wlist:
                j2 = v - 1
                while j2 < total and not kept[j2]:
                    j2 += 1
                new_wait_val[(id(wins), widx)] = pref[j2 + 1]
            for (ins, _, _), k in zip(incs, kept):
                if not k:
                    strip_insts.add(id(ins))
            if ep is big:
                for (rins, uidx, _) in rebases:
                    new_upd_val[(id(rins), uidx)] = pref[-1]

        # apply: rebuild sync lists (element mutation does not persist)
        for ins in insts:
            si = ins.sync_info
            if si is None:
                continue
            if any((id(ins), widx) in new_wait_val
                   for widx in range(len(si.on_wait))):
                si.on_wait = [
                    mybir.SyncWait(
                        sync_type=w.sync_type, id=w.id, ant_name=w.ant_name,
                        wait_mode=w.wait_mode,
                        wait_value=new_wait_val.get((id(ins), widx),
                                                    w.wait_value),
                        wait_reg=w.wait_reg,
                    )
                    for widx, w in enumerate(si.on_wait)
                ]
            if id(ins) in strip_insts:
                keep = [u for u in si.on_update
                        if not (u.sync_type == "semaphore" and u.id == sid
                                and u.update_mode == "sem-inc")]
                stripped += len(si.on_update) - len(keep)
                si.on_update = keep
            elif any((id(ins), uidx) in new_upd_val
                     for uidx in range(len(si.on_update))):
                si.on_update = [
                    mybir.SyncUpdate(
                        sync_type=u.sync_type, id=u.id, ant_name=u.ant_name,
                        update_mode=u.update_mode,
                        update_value=new_upd_val.get((id(ins), uidx),
                                                     u.update_value),
                        update_reg=getattr(u, "update_reg", None),
                    )
                    for uidx, u in enumerate(si.on_update)
                ]
    print(f"strip_midchain_incs: stripped {stripped}")
    return nc


def build_program(with_fast: bool = False, N=None,
                  num_devices=N_CORES, reps: int = 1,
                  strip_incs: bool = True, thin_stops: bool = False):
    """Per-core SPMD program: out^T[vb*128, tokens] = w2_shard @ ht, with
    fp8 DoubleRow 3-term accumulation.

    N defaults to the module's active (deduplicated, padded) token count.
    reps>1 wraps the body in a For_i hardware loop, unrolled x2 over two
    ht buffer sets (reps must be even in that case)."""
    if N is None:
        N = _N_ACTIVE
    CW = N // NCHUNK
    assert N % NCHUNK == 0 and CW % 4 == 0 and 2 * CW <= 512, N
    nc = bacc.Bacc("TRN2", target_bir_lowering=False, debug=False,
                   num_devices=num_devices)
    f8 = mybir.dt.float8e4
    bf = mybir.dt.bfloat16
    f32 = mybir.dt.float32
    DR = mybir.MatmulPerfMode.DoubleRow

    # hth/htl: [kt, p, i, n] with logical d = kt*256 + i*128 + p
    hth_d = nc.dram_tensor("hth", [KT, 128, 2, N], f8, kind="ExternalInput").ap()
    htl_d = nc.dram_tensor("htl", [KT, 128, 2, N], f8, kind="ExternalInput").ap()
    # w2p: [vb*128+p, ver, kt, i, m] = w2_ver[vb*128+m, kt*256+i*128+p]
    w2p_d = nc.dram_tensor("w2p", [VB * 128, 2, KT, 2, 128], f8,
                           kind="ExternalInput").ap()
    out_d = nc.dram_tensor("out", [VB * 128, N], bf, kind="ExternalOutput").ap()

    if reps > 1:
        assert reps % 2 == 0, reps
        parities = (0, 1)
        trip = reps // 2
    else:
        parities = (0,)
        trip = 1

    with tile.TileContext(nc) as tc:
        with (
            tc.tile_pool(name="persist", bufs=1) as persist,
            tc.tile_pool(name="w2s", bufs=4) as w2s,
            tc.tile_pool(name="ostage", bufs=8) as ostage,
            tc.tile_pool(name="psum", bufs=8, space="PSUM") as psum,
        ):
          with (tc.For_i(0, trip, 1) if trip > 1
                else __import__("contextlib").nullcontext()):
            for p in parities:
                # resident h^T tiles: [128, 2, N] per (ktile, hi/lo)
                hth = []
                htl = []
                for kt in range(KT):
                    t = persist.tile([128, 2, N], f8, tag=f"hth{kt}p{p}")
                    nc.sync.dma_start(t[:], hth_d[kt])
                    hth.append(t)
                    if GH_KT > 0:
                        t = persist.tile([128, 2, N], f8, tag=f"htl{kt}p{p}")
                        nc.sync.dma_start(t[:], htl_d[kt])
                        htl.append(t)

                drain_eng = [nc.vector, nc.scalar]
                for vb in range(VB):
                    w2c = w2s.tile([128, 2, KT, 2, 128], f8, tag="w2c")
                    nc.sync.dma_start(w2c[:], w2p_d[vb * 128:(vb + 1) * 128])
                    for th in range(2):
                        pss = [psum.tile([128, CW], f32, tag="ps",
                                         name=f"ps{p}_{vb}_{th}_{n}")
                               for n in range(8)]
                        # per-ktile term list: (moving, w_version)
                        chains = []
                        for kt in range(KT):
                            terms = [(hth[kt], 0)]
                            if kt < GH_KT:
                                terms.append((htl[kt], 0))
                            if kt < GW_KT:
                                terms.append((hth[kt], 1))
                            chains.append(terms)
                        n_terms = sum(len(t) for t in chains)
                        ti = 0
                        for kt in range(KT):
                            for (mv, wver) in chains[kt]:
                                ti += 1
                                for c in range(8):
                                    tok = (th * 8 + c) * CW
                                    nc.tensor.matmul(
                                        pss[c][:],
                                        w2c[:, wver, kt, :, :],
                                        mv[:, :, tok:tok + CW],
                                        start=(ti == 1), stop=(ti == n_terms),
                                        perf_mode=DR,
                                    )
                        for c in range(8):
                            ot = ostage.tile([128, CW], bf, tag="ot")
                            eng = drain_eng[c % 2]
                            if eng is nc.vector:
                                eng.tensor_copy(ot[:], pss[c][:])
                            else:
                                eng.copy(ot[:], pss[c][:])
                            tok = (th * 8 + c) * CW
                            nc.sync.dma_start(
                                out_d[vb * 128:(vb + 1) * 128,
                                      tok:tok + CW],
                                ot[:],
                            )

    nc.compile()
    dedup_ldweights(nc)
    if strip_incs:
        strip_midchain_incs(nc, thin_stops=thin_stops)
    return nc


def _routing_host(x, embed, expert_mu, expert_charge):
    """fp32 host replica of the routing math (same underflow semantics as
    the jax fp32 reference).  Returns (top_idx, top_w, h)."""
    h = embed[x.reshape(-1)].astype(np.float32)                    # (N, D)
    sq = (
        np.sum(h * h, axis=1, keepdims=True)
        + np.sum(expert_mu * expert_mu, axis=1)[None, :]
        - 2.0 * (h @ expert_mu.T)
    ).astype(np.float32)
    kern = np.exp(-np.maximum(sq, 0.0) / np.float32(2.0 * SIGMA ** 2),
                  dtype=np.float32)
    scores = kern * expert_charge[None, :].astype(np.float32)
    mean = scores.mean(axis=0, dtype=np.float32)
    # jax.lax.top_k: descending by value, ties broken by lower index
    top_idx = np.lexsort((np.arange(mean.shape[0]), -mean))[:TOP_K]
    return top_idx, scores[:, top_idx], h


def _pack_ht(a, npad):
    """(U<=npad, 1024) fp8-valued fp32 -> [KT, 128, 2, npad] fp8 with
    [kt, p, i, n] = a[n, kt*256 + i*128 + p]."""
    ap = np.zeros((npad, D), dtype=np.float32)
    ap[:a.shape[0]] = a
    t = ap.T.reshape(KT, 2, 128, npad).transpose(0, 2, 1, 3)
    return np.ascontiguousarray(t).astype(E4M3)


def _pack_w2(wv, wl):
    """Two (6400, 1024) fp8-valued fp32 arrays -> [6400, 2, KT, 2, 128] fp8
    with [vb*128+p, ver, kt, i, m] = w_ver[vb*128+m, kt*256+i*128+p]."""
    packed = []
    for a in (wv, wl):
        t = a.T.reshape(KT, 2, 128, VB, 128).transpose(3, 2, 0, 1, 4)
        packed.append(t)                     # [vb, p, kt, i, m]
    st = np.stack(packed, axis=2)            # [vb, p, ver, kt, i, m]
    return np.ascontiguousarray(st).reshape(VB * 128, 2, KT, 2, 128).astype(E4M3)


def prepare_inputs(x, embed, fast_w1, fast_w2, expert_mu, expert_w,
                   expert_charge):
    """Host-side shard prep. Returns (with_fast, in_maps)."""
    x = np.asarray(x).astype(np.int64).reshape(-1)
    embed = np.asarray(embed, dtype=np.float32)
    fast_w1 = np.asarray(fast_w1, dtype=np.float32)
    fast_w2 = np.asarray(fast_w2, dtype=np.float32)
    expert_mu = np.asarray(expert_mu, dtype=np.float32)
    expert_charge = np.asarray(expert_charge, dtype=np.float32)

    top_idx, top_w, h = _routing_host(x, embed, expert_mu, expert_charge)

    # logits are a per-token-id function, so compute unique token ids only
    # and expand duplicate rows on the host afterwards.
    global _N_ACTIVE, _CW, _LAST_INV
    uniq, inv = np.unique(x, return_inverse=True)
    cw = -(-uniq.size // NCHUNK)
    cw += (-cw) % 4                  # 4-byte-aligned fp8 chunk rows
    npad = NCHUNK * cw
    _N_ACTIVE = npad
    _CW = cw
    _LAST_INV = inv

    # h_merged = relu(h @ W1^T) (+ slow term if the underflow identity ever
    # failed -- it cannot for well-formed inputs, but stay exact anyway)
    h_u = embed[uniq]
    hm = np.maximum(h_u @ fast_w1.T, 0.0).astype(np.float32)
    if np.any(top_w):  # pragma: no cover - degenerate-input safety net
        expert_w = np.asarray(expert_w, dtype=np.float32)
        slow = np.zeros_like(hm)
        for k in range(TOP_K):
            slow += top_w[:, k:k + 1] * (h_u @ expert_w[top_idx[k]].T)
        hm = hm + np.float32(1.0 - FAST_RATIO) * slow

    # hi+lo e4m3 split with error feedback (scales are powers of two)
    hm16 = hm * SH
    hm_hi = hm16.astype(E4M3).astype(np.float32)
    hm_lo = (hm16 - hm_hi).astype(E4M3).astype(np.float32)
    hth = _pack_ht(hm_hi, npad)
    htl = _pack_ht(hm_lo, npad)

    w2tb = fast_w2 * SW                                            # (V, D)
    in_maps = []
    for c in range(N_CORES):
        sh = np.zeros((VB * 128, D), dtype=np.float32)
        lo = c * VS
        hi = min((c + 1) * VS, V)
        sh[:hi - lo] = w2tb[lo:hi]
        w_hi = sh.astype(E4M3).astype(np.float32)
        w_lo = (sh - w_hi).astype(E4M3).astype(np.float32)
        in_maps.append({"hth": hth, "htl": htl,
                        "w2p": _pack_w2(w_hi, w_lo)})
    return False, in_maps


def kernel(**inputs) -> np.ndarray:
    with_fast, in_maps = prepare_inputs(**inputs)
    key = (with_fast, _N_ACTIVE)
    if key not in _prog_cache:
        _prog_cache[key] = build_program(with_fast)
    nc = _prog_cache[key]
    res = run_bass_kernel_spmd(nc, in_maps, core_ids=list(range(N_CORES)))
    # per-core output is transposed logits (VB*128, npad) bf16 at scale
    # SH*SW; trim pad, stack shards, rescale, transpose, expand duplicates
    shards = [res.results[c]["out"][:VS] for c in range(N_CORES)]
    full_t = np.concatenate(shards, axis=0)[:V]      # (V, npad) bf16
    logits_u = full_t.T.astype(np.float32) * INV_S   # (npad, V)
    return np.ascontiguousarray(logits_u[_LAST_INV])
